# revision 1
# baseline (speedup 1.0000x reference)
"""AUGRU cell (attention-scaled GRU update) on 8 Trainium2 NeuronCores.

Data-parallel: batch B=65536 sharded 8 ways (8192 rows/core); gate weights
replicated.  Per core:

  gates_x = x @ W_x.T + b_x          (8192,384)
  gates_h = h @ W_h.T + b_h
  u = sigmoid(.. u block ..); r = sigmoid(.. r block ..)
  h_tilde = tanh(x_c + r * h_c)
  h_new = h_prev + att*u*(h_tilde - h_prev)

v8 design — gate-major layout, host-staged transposed operands:
  - each core receives xT/hT = x/h shard transposed to [I, rows] (a host
    layout/sharding choice; the contraction needs I on partitions either
    way) and the output is produced transposed, flipped back on the host.
  - gates live in PSUM as [gate_type][128, 512]: U/R/Cx/Ch banks.  Two
    accumulating fp32r matmuls for U and R, one each for Cx/Ch; weights
    transposed once at setup on the PE and kept fp32r (fp32-rate 1 cyc/row
    at N>=256, fp32-class accuracy).
  - biases are per-partition in this layout: sigmoid takes them via the
    ACT bias operand; the candidate path folds them into the two DVE
    scalar_tensor_tensor ops.  No bias matmuls, no device transposes,
    no PSUM round-trip copies, no casts on the matmul path.
  - epilogue: u/r/tanh outputs bf16; blend (t-h)*u*att in packed bf16 DVE
    ops; d and the final add against fp32 hT on GPSIMD keep h_prev exact.
"""

import sys

sys.path.insert(0, "/opt/trn_rl_repo")

import os
from contextlib import ExitStack

import numpy as np

import concourse.bass as bass
import concourse.tile as tile
from concourse import bacc, mybir
from concourse.bass_utils import run_bass_kernel_spmd

F32 = mybir.dt.float32
F32R = mybir.dt.float32r
BF16 = mybir.dt.bfloat16
AF = mybir.ActivationFunctionType
OP = mybir.AluOpType

B = 65536
NCORES = 8
BL = int(os.environ.get("AUGRU_BL", B // NCORES))  # 8192 rows per core
I = 128
H = 128
G3 = 3 * H
P = 128
ROWS = 512  # batch rows per group (one fp32 PSUM bank per gate type)
NGROUPS = BL // ROWS

# PSUM banks per group: 0 = U, 1 = R, 2 = Cx, 3 = Ch   (each [128, 512])


def build_program():
    nc = bacc.Bacc("TRN2", target_bir_lowering=False, debug=False)

    xT_d = nc.dram_tensor("xT", [I, BL], F32, kind="ExternalInput").ap()
    hT_d = nc.dram_tensor("hT", [H, BL], F32, kind="ExternalInput").ap()
    a_d = nc.dram_tensor("att_score", [BL], F32, kind="ExternalInput").ap()
    wx_d = nc.dram_tensor("wxT", [I, G3], F32, kind="ExternalInput").ap()
    bx_d = nc.dram_tensor("b_x", [G3], F32, kind="ExternalInput").ap()
    wh_d = nc.dram_tensor("whT", [H, G3], F32, kind="ExternalInput").ap()
    bh_d = nc.dram_tensor("b_h", [G3], F32, kind="ExternalInput").ap()
    o_d = nc.dram_tensor("h_newT", [H, BL], F32, kind="ExternalOutput").ap()

    with tile.TileContext(nc) as tc, ExitStack() as ctx:
        consts = ctx.enter_context(tc.tile_pool(name="consts", bufs=1))
        io = ctx.enter_context(tc.tile_pool(name="io", bufs=8))
        ep = ctx.enter_context(tc.tile_pool(name="ep", bufs=6))
        pg = ctx.enter_context(tc.tile_pool(name="pg", bufs=2, space="PSUM"))

        # ---------------- one-time setup ----------------
        # weights arrive host-transposed [I, 3*128]; DMA straight into the
        # fp32r stationary tile (no device transposes, no identity)
        wT = consts.tile([P, 6, P], F32R, tag="wT")  # [xu, xr, xc, hu, hr, hc]
        nc.sync.dma_start(
            wT[:, 0:3, :], wx_d.rearrange("i (b g) -> i b g", b=3).bitcast(F32R)
        )
        nc.sync.dma_start(
            wT[:, 3:6, :], wh_d.rearrange("i (b g) -> i b g", b=3).bitcast(F32R)
        )

        # per-partition bias columns [128, 1]: b_u+b_hu | b_r+b_hr | b_xc | b_hc
        bxc = consts.tile([P, 3], F32, tag="bxc")
        nc.sync.dma_start(bxc, bx_d.rearrange("(b p) -> p b", p=P))
        bhc = consts.tile([P, 3], F32, tag="bhc")
        nc.sync.dma_start(bhc, bh_d.rearrange("(b p) -> p b", p=P))
        bcol = consts.tile([P, 4], F32, tag="bcol")
        nc.vector.tensor_tensor(bcol[:, 0:2], bxc[:, 0:2], bhc[:, 0:2], OP.add)
        nc.vector.tensor_copy(bcol[:, 2:3], bxc[:, 2:3])
        nc.vector.tensor_copy(bcol[:, 3:4], bhc[:, 2:3])

        # att broadcast to all partitions (GPSIMD), then cast to bf16
        att1 = consts.tile([1, BL], F32R, tag="att1")
        nc.sync.dma_start(att1, a_d.unsqueeze(0).bitcast(F32R))
        ones_f = consts.tile([1, P], F32, tag="ones_f")
        nc.vector.memset(ones_f, 1.0)
        ones = consts.tile([1, P], F32R, tag="ones")
        nc.vector.tensor_copy(ones, ones_f)
        attb = consts.tile([P, BL], BF16, tag="attb")

        # ---------------- pipelined main loop ----------------
        stA = [None] * (NGROUPS + 4)
        stB = [None] * (NGROUPS + 4)
        stC = [None] * (NGROUPS + 4)

        def stage_a(g):
            b0 = g * ROWS
            xs = io.tile([P, ROWS], F32R, tag="xs")
            nc.sync.dma_start(xs, xT_d[:, b0 : b0 + ROWS].bitcast(F32R))
            hs = io.tile([P, ROWS], F32R, tag="hs")
            nc.sync.dma_start(hs, hT_d[:, b0 : b0 + ROWS].bitcast(F32R))
            # att broadcast chunk for this group: a K=1 matmul that fills PE
            # idle during the pipeline ramp, well ahead of e2(g)'s need
            ps = pg.tile([P, ROWS], F32, tag=("gR" if g % 2 else "gCx"))
            nc.tensor.matmul(
                ps[:, :], lhsT=ones,
                rhs=att1[:, b0 : b0 + ROWS],
                start=True, stop=True,
            )
            dst = attb[:, b0 : b0 + ROWS]
            if g % 2:
                nc.scalar.copy(dst, ps[:, :])
            else:
                nc.vector.tensor_copy(dst, ps[:, :])
            return xs, hs

        def stage_b(g):
            xs, hs = stA[g]
            xr = xs
            hr = hs
            gU = pg.tile([P, ROWS], F32, tag="gU")
            gR = pg.tile([P, ROWS], F32, tag="gR")
            gCx = pg.tile([P, ROWS], F32, tag="gCx")
            gCh = pg.tile([P, ROWS], F32, tag="gCh")
            nc.tensor.matmul(gU, lhsT=wT[:, 0, :], rhs=xr, start=True, stop=False)
            nc.tensor.matmul(gR, lhsT=wT[:, 1, :], rhs=xr, start=True, stop=False)
            nc.tensor.matmul(gCx, lhsT=wT[:, 2, :], rhs=xr, start=True, stop=True)
            nc.tensor.matmul(gCh, lhsT=wT[:, 5, :], rhs=hr, start=True, stop=True)
            nc.tensor.matmul(gU, lhsT=wT[:, 3, :], rhs=hr, start=False, stop=True)
            nc.tensor.matmul(gR, lhsT=wT[:, 4, :], rhs=hr, start=False, stop=True)
            return gU, gR, gCx, gCh

        def stage_c(g):
            xs, hs = stA[g]
            gU, gR, gCx, gCh = stB[g]
            r = ep.tile([P, ROWS], BF16, tag="r")
            nc.scalar.activation(r, gR, AF.Sigmoid, bias=bcol[:, 1:2])
            u = ep.tile([P, ROWS], BF16, tag="u")
            nc.scalar.activation(u, gU, AF.Sigmoid, bias=bcol[:, 0:1])
            m = ep.tile([P, ROWS], F32, tag="m")
            nc.vector.scalar_tensor_tensor(
                m, in0=gCh, scalar=bcol[:, 3:4], in1=r,
                op0=OP.add, op1=OP.mult,
            )
            pre = ep.tile([P, ROWS], F32, tag="pre")
            nc.vector.scalar_tensor_tensor(
                pre, in0=gCx, scalar=bcol[:, 2:3], in1=m,
                op0=OP.add, op1=OP.add,
            )
            tb = ep.tile([P, ROWS], BF16, tag="tb")
            nc.scalar.activation(tb, pre, AF.Tanh)
            d = ep.tile([P, ROWS], BF16, tag="d")
            nc.gpsimd.tensor_tensor(d, tb, hs.bitcast(F32), OP.subtract)
            e1 = ep.tile([P, ROWS], BF16, tag="e1")
            nc.vector.tensor_tensor(e1, d, u, OP.mult)
            e2 = ep.tile([P, ROWS], BF16, tag="e2")
            nc.vector.tensor_tensor(
                e2, e1, attb[:, g * ROWS : (g + 1) * ROWS], OP.mult
            )
            return e2

        def stage_c2(g):
            b0 = g * ROWS
            xs, hs = stA[g]
            e2 = stC[g]
            ho = ep.tile([P, ROWS], F32, tag="ho")
            nc.gpsimd.tensor_tensor(ho, e2, hs.bitcast(F32), OP.add)
            # store from the GPSIMD queue: it directly follows ho there, so
            # the sync queue (loads) never head-of-line blocks on epilogues
            nc.gpsimd.dma_start(o_d[:, b0 : b0 + ROWS], ho)

        for k in range(NGROUPS + 4):
            if k < NGROUPS:
                stA[k] = stage_a(k)
            if 2 <= k < NGROUPS + 2:
                stB[k - 2] = stage_b(k - 2)
            if k >= 4:
                stage_c2(k - 4)
            if 3 <= k < NGROUPS + 3:
                stC[k - 3] = stage_c(k - 3)

    nc.compile()
    return nc


_NC_CACHE = []


def _get_nc():
    if not _NC_CACHE:
        _NC_CACHE.append(build_program())
    return _NC_CACHE[0]


def kernel(x, h_prev, att_score, W_x, b_x, W_h, b_h, **_unused):
    x = np.asarray(x, dtype=np.float32)
    h_prev = np.asarray(h_prev, dtype=np.float32)
    att_score = np.ascontiguousarray(np.asarray(att_score, dtype=np.float32))
    W_x = np.ascontiguousarray(np.asarray(W_x, dtype=np.float32))
    b_x = np.ascontiguousarray(np.asarray(b_x, dtype=np.float32))
    W_h = np.ascontiguousarray(np.asarray(W_h, dtype=np.float32))
    b_h = np.ascontiguousarray(np.asarray(b_h, dtype=np.float32))

    nc = _get_nc()
    in_maps = []
    for c in range(NCORES):
        s = slice(c * BL, (c + 1) * BL)
        in_maps.append(
            {
                "xT": np.ascontiguousarray(x[s].T),
                "hT": np.ascontiguousarray(h_prev[s].T),
                "att_score": np.ascontiguousarray(att_score[s]),
                "wxT": np.ascontiguousarray(W_x.T),
                "b_x": b_x,
                "whT": np.ascontiguousarray(W_h.T),
                "b_h": b_h,
            }
        )
    res = run_bass_kernel_spmd(nc, in_maps, list(range(NCORES)))
    out = np.concatenate(
        [np.ascontiguousarray(res.results[c]["h_newT"].T) for c in range(NCORES)],
        axis=0,
    )
    return out



# revision 4
# speedup vs baseline: 1.4386x; 1.4386x over previous
"""AUGRU cell (attention-scaled GRU update) on 8 Trainium2 NeuronCores.

Data-parallel: batch B=65536 sharded 8 ways (8192 rows/core); gate weights
replicated.  Per core (gate-major layout, batch on the free axis):

  gates_x = x @ W_x.T + b_x
  gates_h = h @ W_h.T + b_h
  u = sigmoid(U); r = sigmoid(R); t = tanh(Cx + r*Ch)
  h_new = h + att*u*(t - h)

v9 design — bf16 wire format + engine rebalance:
  - all wire tensors bf16: xT/hT/attb in, h_newT out (host up/down-casts);
    att is pre-broadcast to [128, BL] on the host so the device never pays
    a partition-broadcast.
  - matmuls bf16 (same PE rate as fp32r, half the SBUF/DMA bytes).  Per
    group of 512 batch cols: 2 K=1 bias-prefill matmuls seed the U|R PSUM
    banks so ONE merged sigmoid [128,1024] (no per-gate bias operand
    needed) replaces two; Cx bank is left open and an identity matmul
    accumulates m = (Ch+bCh)*r into it, so tanh reads PSUM directly with
    its bias via the ACT bias operand.  No stt2 pass on DVE.
  - epilogue split: DVE does m(stt), u_att, q, ho; GPSIMD does d = t-h;
    ACT does merged sigmoid + tanh.  PE ~1.9us/group is the design wall.
  - DMA chunked 4 groups per transfer (512KB) to amortize the ~0.6us
    per-dma issue cost on the sync queue; output staged and stored per
    chunk from the sync queue as well.
"""

import sys

sys.path.insert(0, "/opt/trn_rl_repo")

import os
from contextlib import ExitStack

import numpy as np
import ml_dtypes

import concourse.bass as bass
import concourse.tile as tile
from concourse import bacc, mybir
from concourse.bass_utils import run_bass_kernel_spmd

F32 = mybir.dt.float32
BF16 = mybir.dt.bfloat16
AF = mybir.ActivationFunctionType
OP = mybir.AluOpType
BFNP = ml_dtypes.bfloat16

B = 65536
NCORES = 8
BL = B // NCORES  # 8192 rows per core
I = 128
H = 128
G3 = 3 * H
P = 128
ROWS = 512  # batch rows per group (one fp32 PSUM bank per gate)
NGROUPS = BL // ROWS  # 16
CHG = 4  # groups per DMA chunk
CH = CHG * ROWS  # 2048 cols per chunk
NCHUNKS = NGROUPS // CHG  # 4


def build_program():
    nc = bacc.Bacc("TRN2", target_bir_lowering=False, debug=False)

    xT_d = nc.dram_tensor("xT", [I, BL], BF16, kind="ExternalInput").ap()
    hT_d = nc.dram_tensor("hT", [H, BL], BF16, kind="ExternalInput").ap()
    ab_d = nc.dram_tensor("attb", [P, BL], BF16, kind="ExternalInput").ap()
    wx_d = nc.dram_tensor("wxT", [I, 3, P], BF16, kind="ExternalInput").ap()
    wh_d = nc.dram_tensor("whT", [H, 3, P], BF16, kind="ExternalInput").ap()
    bc_d = nc.dram_tensor("bcol", [P, 4], F32, kind="ExternalInput").ap()
    bu_d = nc.dram_tensor("burT", [1, 2 * P], BF16, kind="ExternalInput").ap()
    id_d = nc.dram_tensor("ident", [P, P], BF16, kind="ExternalInput").ap()
    o_d = nc.dram_tensor("h_newT", [H, BL], BF16, kind="ExternalOutput").ap()

    with tile.TileContext(nc) as tc, ExitStack() as ctx:
        consts = ctx.enter_context(tc.tile_pool(name="consts", bufs=1))
        io = ctx.enter_context(tc.tile_pool(name="io", bufs=3))
        og = ctx.enter_context(tc.tile_pool(name="og", bufs=2))
        ep = ctx.enter_context(tc.tile_pool(name="ep", bufs=3))
        pur = ctx.enter_context(tc.tile_pool(name="pur", bufs=2, space="PSUM"))
        pcx = ctx.enter_context(tc.tile_pool(name="pcx", bufs=2, space="PSUM"))
        pch = ctx.enter_context(tc.tile_pool(name="pch", bufs=2, space="PSUM"))

        # ---------------- one-time setup ----------------
        wT = consts.tile([P, 6, P], BF16, tag="wT")  # [xu, xr, xc, hu, hr, hc]
        nc.sync.dma_start(wT[:, 0:3, :], wx_d)
        nc.sync.dma_start(wT[:, 3:6, :], wh_d)
        bcol = consts.tile([P, 4], F32, tag="bcol")  # [bU, bR, bCx, bCh]
        nc.sync.dma_start(bcol, bc_d)
        burT = consts.tile([1, 2 * P], BF16, tag="burT")  # [bU | bR] row
        nc.sync.dma_start(burT, bu_d)
        ident = consts.tile([P, P], BF16, tag="ident")
        nc.sync.dma_start(ident, id_d)
        ones_f = consts.tile([1, ROWS], F32, tag="ones_f")
        nc.vector.memset(ones_f, 1.0)
        ones = consts.tile([1, ROWS], BF16, tag="ones")
        nc.vector.tensor_copy(ones, ones_f)

        xs = [None] * NCHUNKS
        hs = [None] * NCHUNKS
        ab = [None] * NCHUNKS
        oc = [None] * NCHUNKS
        stB = [None] * NGROUPS
        stC = [None] * NGROUPS

        def stage_a(c):
            x = io.tile([P, CH], BF16, tag="xs")
            nc.sync.dma_start(x, xT_d[:, c * CH : (c + 1) * CH])
            h = io.tile([P, CH], BF16, tag="hs")
            nc.sync.dma_start(h, hT_d[:, c * CH : (c + 1) * CH])
            a = io.tile([P, CH], BF16, tag="ab")
            nc.sync.dma_start(a, ab_d[:, c * CH : (c + 1) * CH])
            o = og.tile([P, CH], BF16, tag="oc")
            xs[c], hs[c], ab[c], oc[c] = x, h, a, o

        def stage_b(g):
            c, qi = g // CHG, g % CHG
            sl = slice(qi * ROWS, (qi + 1) * ROWS)
            xg, hg = xs[c][:, sl], hs[c][:, sl]
            gUR = pur.tile([P, 2, ROWS], F32, tag="gUR")
            gCx = pcx.tile([P, ROWS], F32, tag="gCx")
            gCh = pch.tile([P, ROWS], F32, tag="gCh")
            # bias prefill for U and R banks (K=1 outer products)
            nc.tensor.matmul(gUR[:, 0, :], lhsT=burT[:, 0:P], rhs=ones, start=True, stop=False)
            nc.tensor.matmul(gUR[:, 1, :], lhsT=burT[:, P : 2 * P], rhs=ones, start=True, stop=False)
            nc.tensor.matmul(gUR[:, 0, :], lhsT=wT[:, 0, :], rhs=xg, start=False, stop=False)
            nc.tensor.matmul(gUR[:, 1, :], lhsT=wT[:, 1, :], rhs=xg, start=False, stop=False)
            nc.tensor.matmul(gCx, lhsT=wT[:, 2, :], rhs=xg, start=True, stop=False)  # stays open
            nc.tensor.matmul(gCh, lhsT=wT[:, 5, :], rhs=hg, start=True, stop=True)
            nc.tensor.matmul(gUR[:, 0, :], lhsT=wT[:, 3, :], rhs=hg, start=False, stop=True)
            nc.tensor.matmul(gUR[:, 1, :], lhsT=wT[:, 4, :], rhs=hg, start=False, stop=True)
            return gUR, gCx, gCh

        def stage_c(g):
            c, qi = g // CHG, g % CHG
            sl = slice(qi * ROWS, (qi + 1) * ROWS)
            gUR, gCx, gCh = stB[g]
            ur = ep.tile([P, 2, ROWS], BF16, tag="ur")  # [u | r]
            nc.scalar.activation(ur, gUR, AF.Sigmoid)
            m = ep.tile([P, ROWS], BF16, tag="m")
            nc.vector.scalar_tensor_tensor(
                m, in0=gCh, scalar=bcol[:, 3:4], in1=ur[:, 1, :],
                op0=OP.add, op1=OP.mult,
            )
            ua = ep.tile([P, ROWS], BF16, tag="ua")
            nc.vector.tensor_tensor(ua, ur[:, 0, :], ab[c][:, sl], OP.mult)
            # accumulate m into the open Cx bank: pre = gCx + m (bias via tanh)
            nc.tensor.matmul(gCx, lhsT=ident, rhs=m, start=False, stop=True)
            tb = ep.tile([P, ROWS], BF16, tag="tb")
            nc.scalar.activation(tb, gCx, AF.Tanh, bias=bcol[:, 2:3])
            return ua, tb

        def stage_e(g):
            c, qi = g // CHG, g % CHG
            sl = slice(qi * ROWS, (qi + 1) * ROWS)
            ua, tb = stC[g]
            d = ep.tile([P, ROWS], BF16, tag="d")
            nc.gpsimd.tensor_tensor(d, tb, hs[c][:, sl], OP.subtract)
            q = ep.tile([P, ROWS], BF16, tag="q")
            nc.vector.tensor_tensor(q, d, ua, OP.mult)
            nc.vector.tensor_tensor(oc[c][:, sl], q, hs[c][:, sl], OP.add)
            if qi == CHG - 1:
                nc.sync.dma_start(o_d[:, c * CH : (c + 1) * CH], oc[c])

        for k in range(NGROUPS + 5):
            if k < NGROUPS and k % CHG == 0:
                stage_a(k // CHG)
            if 2 <= k < NGROUPS + 2:
                stB[k - 2] = stage_b(k - 2)
            if 3 <= k < NGROUPS + 3:
                stC[k - 3] = stage_c(k - 3)
            if k >= 5:
                stage_e(k - 5)

    nc.compile()
    return nc


_NC_CACHE = []


def _get_nc():
    if not _NC_CACHE:
        _NC_CACHE.append(build_program())
    return _NC_CACHE[0]


def make_in_maps(x, h_prev, att_score, W_x, b_x, W_h, b_h):
    """Shard + stage inputs for the 8 cores (bf16 wire format)."""
    x = np.asarray(x, dtype=np.float32)
    h_prev = np.asarray(h_prev, dtype=np.float32)
    att = np.asarray(att_score, dtype=np.float32)
    W_x = np.asarray(W_x, dtype=np.float32)
    W_h = np.asarray(W_h, dtype=np.float32)
    b_x = np.asarray(b_x, dtype=np.float32)
    b_h = np.asarray(b_h, dtype=np.float32)

    wxT = np.ascontiguousarray(W_x.T.reshape(I, 3, P).astype(BFNP))
    whT = np.ascontiguousarray(W_h.T.reshape(H, 3, P).astype(BFNP))
    bsum = b_x + b_h  # valid for U and R blocks
    bcol = np.stack(
        [bsum[0:P], bsum[P : 2 * P], b_x[2 * P : 3 * P], b_h[2 * P : 3 * P]], axis=1
    ).astype(np.float32)
    burT = np.ascontiguousarray(bsum[0 : 2 * P].reshape(1, 2 * P).astype(BFNP))
    ident = np.eye(P, dtype=BFNP)

    in_maps = []
    for c in range(NCORES):
        s = slice(c * BL, (c + 1) * BL)
        attb = np.broadcast_to(att[s].astype(BFNP), (P, BL))
        in_maps.append(
            {
                "xT": np.ascontiguousarray(x[s].T.astype(BFNP)),
                "hT": np.ascontiguousarray(h_prev[s].T.astype(BFNP)),
                "attb": np.ascontiguousarray(attb),
                "wxT": wxT,
                "whT": whT,
                "bcol": bcol,
                "burT": burT,
                "ident": ident,
            }
        )
    return in_maps


def kernel(x, h_prev, att_score, W_x, b_x, W_h, b_h, **_unused):
    nc = _get_nc()
    in_maps = make_in_maps(x, h_prev, att_score, W_x, b_x, W_h, b_h)
    res = run_bass_kernel_spmd(nc, in_maps, list(range(NCORES)))
    out = np.concatenate(
        [
            np.asarray(res.results[c]["h_newT"]).astype(np.float32).T
            for c in range(NCORES)
        ],
        axis=0,
    )
    return np.ascontiguousarray(out)
